# revision 14
# baseline (speedup 1.0000x reference)
"""AxialAttention TRN2 Bass kernel (pipelined).

Shapes (hardcoded): x [B=4,T=16,C=256,H=64,W=64] fp32.
N = B*T*H = 4096 lines of [L=64, C=256]; heads=8, d=32.
Sharding: 64 (b,t) blocks -> 8 per core across 8 cores.

Per-core dataflow, per (b,t) block (xT = x[b,t] viewed [C=256, HW=4096]):
  qkT  = w_qk^T @ xT          (fp32r MMs, N=512)   q pre-scaled by 1/sqrt(d)
  v    = xT^T @ w_v           (row-major v)
  per 8-line group g, per psum bank hr (heads hr, hr+4 stacked):
    scoresT = kT^T @ qT        (bf16 packed MMs, start=True; NO bias seed)
    probs0  = exp(scoresT)     (ACT -> bf16 sbuf)
    probs   = probs0 * exp(bias)  (DVE bf16 mul; softmax bias via factoring)
    Z       = Ez^T @ probs     (column sums, psum [8, 512])
    rz      = recip_approx_fast(Z)   (DVE custom op)
    rbc     = Ebc^T @ rz       (broadcast rz over d=32 partitions)
    oT      = v^T @ probs      (bf16 packed MMs)
    oT_sb   = oT * rbc
    outT    = w_proj^T @ oT_sb + b'
Software pipeline (emission order per iteration g):
  PE:     Z(g-1), rbc(g-2), scores(g), attnv(g-2), proj(g-2)
  Scalar: exp(g), proj-evac(g-2)
  DVE:    recip(g-1), rbc-copy(g-2), oT-mul(g-2), ebmul(g)
PSUM: psA=4 (scores), psB=2 (Z/rbc/ops), psM=2 (qkv+proj MMs).
b_k dropped (softmax shift-invariant); b_q folded into q evacuation bias;
b_v folded into b' = b_v @ w_proj + b_proj.
"""

import numpy as np

B, T, C, H, W = 4, 16, 256, 64, 64
HEADS, D = 8, 32
NBT = B * T            # 64 (b,t) blocks
NCORES = 8
BT_PER_CORE = NBT // NCORES  # 8
HW = H * W             # 4096 positions per block
L = W                  # 64
GRP = 8                # lines per attention group
NGRP = H // GRP        # 8 groups per block
GQ = GRP * L           # 512 free columns per group

ST_COLS = 4388


def _build_bass():
    import concourse.bacc as bacc
    import concourse.mybir as mybir
    from concourse.tile import TileContext

    f32 = mybir.dt.float32
    f32r = mybir.dt.float32r
    bf16 = mybir.dt.bfloat16
    AF = mybir.ActivationFunctionType

    nc = bacc.Bacc("TRN2", target_bir_lowering=False, debug=False,
                   num_devices=NCORES)

    x_d = nc.dram_tensor("x", [BT_PER_CORE, C, HW], f32r, kind="ExternalInput").ap()
    st_d = nc.dram_tensor("statics", [128, ST_COLS], f32r, kind="ExternalInput").ap()
    out_d = nc.dram_tensor("out", [BT_PER_CORE, C, HW], f32, kind="ExternalOutput").ap()

    with TileContext(nc) as tc:
        with (
            tc.tile_pool(name="static", bufs=1) as stat,
            tc.tile_pool(name="xt", bufs=2) as pxt,
            tc.tile_pool(name="qk", bufs=4) as pqk,
            tc.tile_pool(name="vsp", bufs=3) as pv,
            tc.tile_pool(name="praw", bufs=1) as praw,
            tc.tile_pool(name="probs", bufs=3) as ppr,
            tc.tile_pool(name="zsb", bufs=2) as pz,
            tc.tile_pool(name="rbc", bufs=2) as prb,
            tc.tile_pool(name="osb", bufs=2) as po,
            tc.tile_pool(name="outsb", bufs=2) as pout,
            tc.tile_pool(name="psA", bufs=4, space="PSUM") as psA,
            tc.tile_pool(name="psB", bufs=2, space="PSUM") as psB,
            tc.tile_pool(name="psM", bufs=2, space="PSUM") as psM,
        ):
            # ---- static loads: one packed DMA ----
            st = stat.tile([128, ST_COLS], f32r, tag="st", name="statics_sb")
            nc.sync.dma_start(out=st, in_=st_d)
            wqk = [st[:, 512 * i:512 * (i + 1)] for i in range(2)]
            wv = [st[:, 1024 + 256 * i:1024 + 256 * (i + 1)] for i in range(2)]
            wp = [[st[:, 1536 + 256 * i + 128 * j:1536 + 256 * i + 128 * (j + 1)]
                   for j in range(2)] for i in range(2)]
            eb_f32 = st[:, 2048:4096]
            bq = st[:, 4096:4098].bitcast(f32)
            bp = st[:, 4098:4100].bitcast(f32)
            ebc = st[0:8, 4100:4356]
            ez = stat.tile([128, 32], bf16, tag="ez", name="ez")
            eb = stat.tile([128, 2048], bf16, tag="eb", name="eb")
            with nc.allow_low_precision(reason="exact constants / bias factor"):
                nc.vector.tensor_copy(ez, st[:, 4356:4388])
                nc.vector.tensor_copy(eb, eb_f32)
            # block-diagonal v for head-pair-packed attn@v:
            # slot (l, hr) at cols 64*(4l+hr): [0:64, 0:32] = v_l,hr,
            # [64:128, 32:64] = v_l,hr+4, other quadrants stay zero.
            v_bd = stat.tile([128, 64 * 4 * L], bf16, tag="vbd", name="v_bd")
            with nc.allow_low_precision(reason="zero init"):
                nc.vector.memset(v_bd, 0.0)

            # ---------------- emission helpers ----------------
            def load_x(bt):
                tiles = [pxt.tile([128, HW], f32r, tag="xt", name="xt")
                         for _ in range(2)]
                for kc in range(2):
                    nc.sync.dma_start(
                        out=tiles[kc][:, :2048],
                        in_=x_d[bt, 128 * kc:128 * (kc + 1), :2048])
                    nc.sync.dma_start(
                        out=tiles[kc][:, 2048:],
                        in_=x_d[bt, 128 * kc:128 * (kc + 1), 2048:])
                return tiles

            def qkvproj(xt):
                # qk projection: qkT [512, 4096] -> bf16 sbuf
                qkT = [pqk.tile([128, HW], bf16, tag="qkT", name="qkT")
                       for _ in range(4)]
                for mc in range(4):
                    for nn in range(8):
                        ps = psM.tile([128, 512], f32, tag="mm", name="psmm")
                        for kc in range(2):
                            nc.tensor.matmul(
                                ps, wqk[kc][:, 128 * mc:128 * (mc + 1)],
                                xt[kc][:, 512 * nn:512 * (nn + 1)],
                                start=(kc == 0), stop=(kc == 1))
                        dst = qkT[mc][:, 512 * nn:512 * (nn + 1)]
                        if mc < 2:  # q: fold b_q (pre-scaled) per-partition
                            nc.scalar.activation(dst, ps, AF.Identity,
                                                 bias=bq[:, mc:mc + 1], scale=1.0)
                        else:       # k: plain copy (b_k dropped)
                            nc.vector.tensor_copy(dst, ps)
                # v projection into span staging, then strided DMAs build the
                # block-diagonal v_bd (slot (l,hr): TL=v_l,hr BR=v_l,hr+4).
                vbd_v = v_bd.rearrange(
                    "p (s j par hr d2) -> p s j par hr d2",
                    s=4, j=8, par=2, hr=4, d2=64)
                for sp in range(4):
                    vsp = pv.tile([128, 2048], bf16, tag="vsp", name="vsp")
                    for j in range(8):
                        pc = 8 * sp + j
                        ps = psM.tile([128, 256], f32, tag="mm", name="psmmv")
                        for kc in range(2):
                            nc.tensor.matmul(
                                ps, xt[kc][:, 128 * pc:128 * (pc + 1)],
                                wv[kc], start=(kc == 0), stop=(kc == 1))
                        if j % 2 == 0:
                            nc.scalar.copy(vsp[:, 256 * j:256 * (j + 1)], ps)
                        else:
                            nc.vector.tensor_copy(vsp[:, 256 * j:256 * (j + 1)], ps)
                    src_v = vsp.rearrange("p (j hh h d) -> p j hh h d",
                                          j=8, hh=2, h=4, d=32)
                    for hr in range(4):
                        nc.sync.dma_start(
                            out=vbd_v[0:64, sp, :, 0, hr, 0:32],
                            in_=src_v[0:64, :, 0, hr, :])
                        nc.sync.dma_start(
                            out=vbd_v[64:128, sp, :, 0, hr, 32:64],
                            in_=src_v[0:64, :, 1, hr, :])
                        nc.sync.dma_start(
                            out=vbd_v[0:64, sp, :, 1, hr, 0:32],
                            in_=src_v[64:128, :, 0, hr, :])
                        nc.sync.dma_start(
                            out=vbd_v[64:128, sp, :, 1, hr, 32:64],
                            in_=src_v[64:128, :, 1, hr, :])
                return qkT

            def scores(g, qkT, S):
                l0 = g * GRP
                sps = [psA.tile([128, GQ], f32, tag="att", name="psatt")
                       for _ in range(4)]
                for li in range(GRP):
                    l = l0 + li
                    for h in range(HEADS):
                        hc, hr = h // 4, h % 4
                        kt = qkT[2 + hc][32 * hr:32 * (hr + 1),
                                         64 * l:64 * (l + 1)]
                        qt = qkT[hc][32 * hr:32 * (hr + 1),
                                     64 * l:64 * (l + 1)]
                        nc.tensor.matmul(
                            sps[hr][64 * hc:64 * (hc + 1),
                                    64 * li:64 * (li + 1)],
                            kt, qt, start=True, stop=True,
                            tile_position=(32 * hr, 64 * hc))
                S[g] = {"sps": sps}

            def exp_emit(g, S):
                sps = S[g]["sps"]
                probs0 = praw.tile([128, 4 * GQ], bf16, tag="praw", name="praw")
                for b in range(4):
                    nc.scalar.activation(
                        probs0[:, GQ * b:GQ * (b + 1)], sps[b], AF.Exp,
                        scale=1.0)
                S[g]["probs0"] = probs0

            def ebmul_emit(g, S):
                probs0 = S[g]["probs0"]
                probs = ppr.tile([128, 4 * GQ], bf16, tag="probs", name="probs")
                with nc.allow_low_precision(reason="softmax weights bf16"):
                    for b in range(4):
                        nc.vector.tensor_mul(
                            probs[:, GQ * b:GQ * (b + 1)],
                            probs0[:, GQ * b:GQ * (b + 1)],
                            eb[:, GQ * b:GQ * (b + 1)])
                S[g]["probs"] = probs

            def z_emit(g, S):
                probs = S[g]["probs"]
                zps = psB.tile([8, GQ], f32, tag="bb", name="psz")
                for b in range(4):
                    nc.tensor.matmul(
                        zps, ez[:, 8 * b:8 * (b + 1)],
                        probs[:, GQ * b:GQ * (b + 1)],
                        start=(b == 0), stop=(b == 3))
                S[g]["zps"] = zps

            def recip_emit(g, S):
                z_f = pz.tile([8, GQ], f32, tag="zf", name="zf")
                z_sb = pz.tile([8, GQ], f32r, tag="z", name="zsb")
                nc.vector.reciprocal_approx_fast(z_f, S[g]["zps"])
                with nc.allow_low_precision(reason="f32r bits are f32"):
                    nc.vector.tensor_copy(z_sb, z_f)
                S[g]["z_sb"] = z_sb

            def rbc_mm_emit(g, S):
                z_r = S[g]["z_sb"]
                rbc = [psB.tile([128, GQ], f32, tag="bb", name="psrbc")
                       for _ in range(2)]
                for c in range(2):
                    nc.tensor.matmul(
                        rbc[c], ebc[:, 128 * c:128 * (c + 1)],
                        z_r, start=True, stop=True)
                S[g]["rbc"] = rbc

            def rbc_copy_emit(g, S):
                rbc_sb = [prb.tile([128, GQ], f32, tag="rbc", name="rbcsb")
                          for _ in range(2)]
                for c in range(2):
                    nc.vector.tensor_copy(rbc_sb[c], S[g]["rbc"][c])
                S[g]["rbc_sb"] = rbc_sb

            def attnv_emit(g, S):
                # head-pair packed: one MM covers heads (hr, hr+4) via the
                # block-diagonal v_bd stationary; probs bank hr already has
                # k_hr | k_hr+4 stacked on partitions.
                l0 = g * GRP
                probs = S[g]["probs"]
                ops = [psB.tile([128, GQ], f32, tag="bb", name="psops")
                       for _ in range(2)]
                for li in range(GRP):
                    l = l0 + li
                    for hr in range(4):
                        vt = v_bd[:, 64 * (4 * l + hr):64 * (4 * l + hr) + 64]
                        pt = probs[:, GQ * hr + 64 * li:GQ * hr + 64 * (li + 1)]
                        nc.tensor.matmul(
                            ops[hr // 2][64 * (hr % 2):64 * (hr % 2) + 64,
                                         64 * li:64 * (li + 1)],
                            vt, pt, start=True, stop=True,
                            tile_position=(0, 64 * (hr % 2)))
                S[g]["ops"] = ops

            def otmul_emit(g, S):
                oT = [po.tile([128, GQ], f32r, tag="oT", name="oT")
                      for _ in range(2)]
                with nc.allow_low_precision(reason="f32r bits are f32"):
                    for c in range(2):
                        nc.vector.tensor_mul(oT[c], S[g]["ops"][c],
                                             S[g]["rbc_sb"][c])
                S[g]["oT"] = oT

            def proj_emit(g, S, bt):
                oT = S[g]["oT"]
                for mc in range(2):
                    ps = psM.tile([128, GQ], f32, tag="mm", name="psproj")
                    for kc in range(2):
                        nc.tensor.matmul(ps, wp[kc][mc], oT[kc],
                                         start=(kc == 0), stop=(kc == 1))
                    osb = pout.tile([128, GQ], f32, tag="out", name="outsb")
                    nc.scalar.activation(osb, ps, AF.Identity,
                                         bias=bp[:, mc:mc + 1], scale=1.0)
                    nc.sync.dma_start(
                        out=out_d[bt, 128 * mc:128 * (mc + 1),
                                  GQ * g:GQ * (g + 1)],
                        in_=osb)
                del S[g]

            def post_head(g, S):
                # PE-early piece of post(g): rbc MMs (before scores of g+2)
                rbc_mm_emit(g, S)

            def post_tail(g, S, bt):
                rbc_copy_emit(g, S)          # DVE
                attnv_emit(g, S)             # PE
                otmul_emit(g, S)             # DVE
                proj_emit(g, S, bt)          # PE + scalar + DMA

            # ---------------- main loop ----------------
            xt_next = load_x(0)
            for bt in range(BT_PER_CORE):
                xt = xt_next
                if bt + 1 < BT_PER_CORE:
                    xt_next = load_x(bt + 1)
                qkT = qkvproj(xt)
                S = {}
                for g in range(NGRP):
                    if g >= 1:
                        z_emit(g - 1, S)          # PE
                    if g >= 2:
                        post_head(g - 2, S)       # PE (rbc MMs)
                    scores(g, qkT, S)             # PE
                    if g >= 1:
                        recip_emit(g - 1, S)      # DVE (early)
                    exp_emit(g, S)                # scalar
                    ebmul_emit(g, S)              # DVE
                    if g >= 2:
                        post_tail(g - 2, S, bt)
                # flush
                post_head(NGRP - 2, S)
                post_tail(NGRP - 2, S, bt)
                z_emit(NGRP - 1, S)
                recip_emit(NGRP - 1, S)
                post_head(NGRP - 1, S)
                post_tail(NGRP - 1, S, bt)
    nc.compile()
    return nc


def _host_inputs(x, relative_bias, w_qkv, b_qkv, w_proj, b_proj):
    scale = D ** -0.5
    wq = w_qkv[:, :C] * scale          # [256, 256]
    wk = w_qkv[:, C:2 * C]
    wv = w_qkv[:, 2 * C:]
    bqv = b_qkv[:C] * scale            # [256]
    bv = b_qkv[2 * C:]
    wqk_full = np.concatenate([wq, wk], axis=1)        # [256, 512]
    wqk = np.stack([wqk_full[:128], wqk_full[128:]]).astype(np.float32)
    wvs = np.stack([wv[:128], wv[128:]]).astype(np.float32)
    # proj weight rows permuted to the attn@v output feature order
    # (head-pair packed): bank0 = h0,h4,h1,h5; bank1 = h2,h6,h3,h7.
    perm = [0, 4, 1, 5, 2, 6, 3, 7]
    wpr = np.concatenate([w_proj[32 * h:32 * (h + 1)] for h in perm], axis=0)
    wp = np.zeros((2, 2, 128, 128), np.float32)
    for kc in range(2):
        for mc in range(2):
            wp[kc, mc] = wpr[128 * kc:128 * (kc + 1),
                             128 * mc:128 * (mc + 1)]
    bq = np.stack([bqv[:128], bqv[128:]], axis=1).astype(np.float32)  # [128,2]
    bpv = bv @ w_proj + b_proj                                       # [256]
    bp = np.stack([bpv[:128], bpv[128:]], axis=1).astype(np.float32)
    # exp(bias) tiles, matching probs layout: bank hr, partition half hh =
    # head h = hr + 4*hh; [k, q] = bias[h].T; tiled 8 lines along free.
    eb = np.zeros((128, 2048), np.float32)
    for hr in range(4):
        for hh in range(2):
            h = hr + 4 * hh
            ebT = np.exp(relative_bias[h].T.astype(np.float64)).astype(np.float32)
            eb[64 * hh:64 * (hh + 1), 512 * hr:512 * (hr + 1)] = \
                np.tile(ebT, (1, 8))
    ez = np.zeros((128, 32), np.float32)
    for b in range(4):
        ez[0:64, 8 * b + 2 * b] = 1        # head b    -> z row 2b
        ez[64:128, 8 * b + 2 * b + 1] = 1  # head b+4  -> z row 2b+1
    ebc = np.zeros((8, 256), np.float32)
    for c in range(2):
        for m in range(4):
            h = perm[4 * c + m]
            zrow = 2 * (h % 4) + (h // 4)
            ebc[zrow, 128 * c + 32 * m:128 * c + 32 * (m + 1)] = 1.0
    st = np.zeros((128, ST_COLS), np.float32)
    st[:, 0:512] = wqk[0]
    st[:, 512:1024] = wqk[1]
    st[:, 1024:1280] = wvs[0]
    st[:, 1280:1536] = wvs[1]
    for i in range(2):
        for j in range(2):
            st[:, 1536 + 256 * i + 128 * j:1536 + 256 * i + 128 * (j + 1)] = wp[i, j]
    st[:, 2048:4096] = eb
    st[:, 4096:4098] = bq
    st[:, 4098:4100] = bp
    st[0:8, 4100:4356] = ebc
    st[:, 4356:4388] = ez
    return dict(statics=st)


def kernel(x, relative_bias, w_qkv, b_qkv, w_proj, b_proj):
    import sys
    if '/opt/trn_rl_repo' not in sys.path:
        sys.path.insert(0, '/opt/trn_rl_repo')
    from concourse.bass_utils import run_bass_kernel_spmd

    x = np.asarray(x, np.float32)
    const = _host_inputs(np.asarray(x, np.float32),
                         np.asarray(relative_bias, np.float32),
                         np.asarray(w_qkv, np.float32),
                         np.asarray(b_qkv, np.float32),
                         np.asarray(w_proj, np.float32),
                         np.asarray(b_proj, np.float32))
    # x [B,T,C,H,W] -> [64, 256, 4096]
    xr = np.ascontiguousarray(x.reshape(NBT, C, HW))
    nc = _build_bass()
    in_maps = []
    for c in range(NCORES):
        m = dict(const)
        m["x"] = np.ascontiguousarray(xr[c * BT_PER_CORE:(c + 1) * BT_PER_CORE])
        in_maps.append(m)
    res = run_bass_kernel_spmd(nc, in_maps, list(range(NCORES)))
    globals()["LAST_RESULT"] = res
    outs = res.results
    out = np.concatenate([o["out"].reshape(BT_PER_CORE, C, HW) for o in outs],
                         axis=0)
    return out.reshape(B, T, C, H, W).astype(np.float32)


# revision 26
# speedup vs baseline: 1.1139x; 1.1139x over previous
"""AxialAttention TRN2 Bass kernel (pipelined).

Shapes (hardcoded): x [B=4,T=16,C=256,H=64,W=64] fp32.
N = B*T*H = 4096 lines of [L=64, C=256]; heads=8, d=32.
Sharding: 64 (b,t) blocks -> 8 per core across 8 cores.

Per-core dataflow, per (b,t) block (xT = x[b,t] viewed [C=256, HW=4096]):
  qkT  = w_qk^T @ xT          (fp32r MMs, N=512)   q pre-scaled by 1/sqrt(d)
  v    = xT^T @ w_v           (row-major v)
  per 8-line group g, per psum bank hr (heads hr, hr+4 stacked):
    scoresT = kT^T @ qT        (bf16 packed MMs, start=True; NO bias seed)
    probs0  = exp(scoresT)     (ACT -> bf16 sbuf)
    probs   = probs0 * exp(bias)  (DVE bf16 mul; softmax bias via factoring)
    Z       = Ez^T @ probs     (column sums, psum [8, 512])
    rz      = recip_approx_fast(Z)   (DVE custom op)
    rbc     = Ebc^T @ rz       (broadcast rz over d=32 partitions)
    oT      = v^T @ probs      (bf16 packed MMs)
    oT_sb   = oT * rbc
    outT    = w_proj^T @ oT_sb + b'
Software pipeline (emission order per iteration g):
  PE:     Z(g-1), rbc(g-2), scores(g), attnv(g-2), proj(g-2)
  Scalar: exp(g), proj-evac(g-2)
  DVE:    recip(g-1), rbc-copy(g-2), oT-mul(g-2), ebmul(g)
PSUM: psA=4 (scores), psB=2 (Z/rbc/ops), psM=2 (qkv+proj MMs).
b_k dropped (softmax shift-invariant); b_q folded into q evacuation bias;
b_v folded into b' = b_v @ w_proj + b_proj.
"""

import numpy as np

B, T, C, H, W = 4, 16, 256, 64, 64
HEADS, D = 8, 32
NBT = B * T            # 64 (b,t) blocks
NCORES = 8
BT_PER_CORE = NBT // NCORES  # 8
HW = H * W             # 4096 positions per block
L = W                  # 64
GRP = 8                # lines per attention group
NGRP = H // GRP        # 8 groups per block
GQ = GRP * L           # 512 free columns per group

ST_COLS = 4388


def _build_bass():
    import concourse.bacc as bacc
    import concourse.mybir as mybir
    from concourse.tile import TileContext

    f32 = mybir.dt.float32
    f32r = mybir.dt.float32r
    bf16 = mybir.dt.bfloat16
    AF = mybir.ActivationFunctionType

    nc = bacc.Bacc("TRN2", target_bir_lowering=False, debug=False,
                   num_devices=NCORES)

    x_d = nc.dram_tensor("x", [BT_PER_CORE, C, HW], f32r, kind="ExternalInput").ap()
    st_d = nc.dram_tensor("statics", [128, ST_COLS], f32r, kind="ExternalInput").ap()
    out_d = nc.dram_tensor("out", [BT_PER_CORE, C, HW], f32, kind="ExternalOutput").ap()

    with TileContext(nc) as tc:
        with (
            tc.tile_pool(name="static", bufs=1) as stat,
            tc.tile_pool(name="xt", bufs=2) as pxt,
            tc.tile_pool(name="qk", bufs=4) as pqk,
            tc.tile_pool(name="vsb", bufs=2) as pv,
            tc.tile_pool(name="praw", bufs=1) as praw,
            tc.tile_pool(name="probs", bufs=3) as ppr,
            tc.tile_pool(name="zsb", bufs=2) as pz,
            tc.tile_pool(name="rbc", bufs=2) as prb,
            tc.tile_pool(name="osb", bufs=2) as po,
            tc.tile_pool(name="outsb", bufs=2) as pout,
            tc.tile_pool(name="psA", bufs=4, space="PSUM") as psA,
            tc.tile_pool(name="psB", bufs=2, space="PSUM") as psB,
            tc.tile_pool(name="psM", bufs=2, space="PSUM") as psM,
        ):
            # ---- static loads: one packed DMA ----
            st = stat.tile([128, ST_COLS], f32r, tag="st", name="statics_sb")
            nc.sync.dma_start(out=st, in_=st_d)
            wqk = [st[:, 512 * i:512 * (i + 1)] for i in range(2)]
            wv = [st[:, 1024 + 256 * i:1024 + 256 * (i + 1)] for i in range(2)]
            wp = [[st[:, 1536 + 256 * i + 128 * j:1536 + 256 * i + 128 * (j + 1)]
                   for j in range(2)] for i in range(2)]
            eb_f32 = st[:, 2048:4096]
            bq = st[:, 4096:4098].bitcast(f32)
            bp = st[:, 4098:4100].bitcast(f32)
            ebc = st[0:8, 4100:4356]
            ez = stat.tile([128, 32], bf16, tag="ez", name="ez")
            eb = stat.tile([128, 2048], bf16, tag="eb", name="eb")
            with nc.allow_low_precision(reason="exact constants / bias factor"):
                nc.vector.tensor_copy(ez, st[:, 4356:4388])
                nc.vector.tensor_copy(eb, eb_f32)


            # ---------------- emission helpers ----------------
            def load_x(bt):
                tiles = [pxt.tile([128, HW], f32r, tag="xt", name="xt")
                         for _ in range(2)]
                for kc in range(2):
                    nc.sync.dma_start(
                        out=tiles[kc][:, :2048],
                        in_=x_d[bt, 128 * kc:128 * (kc + 1), :2048])
                    nc.sync.dma_start(
                        out=tiles[kc][:, 2048:],
                        in_=x_d[bt, 128 * kc:128 * (kc + 1), 2048:])
                return tiles

            def qkvproj(xt):
                qkT = [pqk.tile([128, HW], bf16, tag="qkT", name="qkT")
                       for _ in range(4)]
                # v projection FIRST so the partition-swap DMAs complete
                # before attnv(0) reads v_sw.
                # v (row-major): v_sb [128 = 2 lines, 32 chunks * 256]
                v_sb = pv.tile([128, 8192], bf16, tag="vsb", name="vsb")
                v_sw = pv.tile([128, 8192], bf16, tag="vsw", name="vsw")
                for pc in range(32):
                    ps = psM.tile([128, 256], f32, tag="mm", name="psmmv")
                    for kc in range(2):
                        nc.tensor.matmul(
                            ps, xt[kc][:, 128 * pc:128 * (pc + 1)],
                            wv[kc], start=(kc == 0), stop=(kc == 1))
                    if pc % 2 == 0:
                        nc.scalar.copy(v_sb[:, 256 * pc:256 * (pc + 1)], ps)
                    else:
                        nc.vector.tensor_copy(v_sb[:, 256 * pc:256 * (pc + 1)], ps)
                    if pc % 8 == 7:  # pipelined partition-swap of finished span
                        c0, c1 = 256 * (pc - 7), 256 * (pc + 1)
                        nc.sync.dma_start(out=v_sw[0:64, c0:c1],
                                          in_=v_sb[64:128, c0:c1])
                        nc.sync.dma_start(out=v_sw[64:128, c0:c1],
                                          in_=v_sb[0:64, c0:c1])
                # qk projection: qkT [512, 4096] -> bf16 sbuf
                for mc in range(4):
                    for nn in range(8):
                        ps = psM.tile([128, 512], f32, tag="mm", name="psmm")
                        for kc in range(2):
                            nc.tensor.matmul(
                                ps, wqk[kc][:, 128 * mc:128 * (mc + 1)],
                                xt[kc][:, 512 * nn:512 * (nn + 1)],
                                start=(kc == 0), stop=(kc == 1))
                        dst = qkT[mc][:, 512 * nn:512 * (nn + 1)]
                        if mc < 2:  # q: fold b_q (pre-scaled) per-partition
                            nc.scalar.activation(dst, ps, AF.Identity,
                                                 bias=bq[:, mc:mc + 1], scale=1.0)
                        else:       # k: plain copy (b_k dropped)
                            nc.vector.tensor_copy(dst, ps)
                return qkT, v_sb, v_sw

            def scores(g, qkT, S):
                l0 = g * GRP
                sps = [psA.tile([128, GQ], f32, tag="att", name="psatt")
                       for _ in range(4)]
                for li in range(GRP):
                    l = l0 + li
                    for h in range(HEADS):
                        hc, hr = h // 4, h % 4
                        kt = qkT[2 + hc][32 * hr:32 * (hr + 1),
                                         64 * l:64 * (l + 1)]
                        qt = qkT[hc][32 * hr:32 * (hr + 1),
                                     64 * l:64 * (l + 1)]
                        nc.tensor.matmul(
                            sps[hr][64 * hc:64 * (hc + 1),
                                    64 * li:64 * (li + 1)],
                            kt, qt, start=True, stop=True,
                            tile_position=(32 * hr, 64 * hc))
                S[g] = {"sps": sps}

            def exp_emit(g, S):
                sps = S[g]["sps"]
                probs0 = praw.tile([128, 4 * GQ], bf16, tag="praw", name="praw")
                for b in range(4):
                    nc.scalar.activation(
                        probs0[:, GQ * b:GQ * (b + 1)], sps[b], AF.Exp,
                        scale=1.0)
                S[g]["probs0"] = probs0

            def ebmul_emit(g, S):
                probs0 = S[g]["probs0"]
                probs = ppr.tile([128, 4 * GQ], bf16, tag="probs", name="probs")
                with nc.allow_low_precision(reason="softmax weights bf16"):
                    for b in range(4):
                        nc.vector.tensor_mul(
                            probs[:, GQ * b:GQ * (b + 1)],
                            probs0[:, GQ * b:GQ * (b + 1)],
                            eb[:, GQ * b:GQ * (b + 1)])
                S[g]["probs"] = probs

            def z_emit(g, S):
                probs = S[g]["probs"]
                zps = psB.tile([8, GQ], f32, tag="bb", name="psz")
                for b in range(4):
                    nc.tensor.matmul(
                        zps, ez[:, 8 * b:8 * (b + 1)],
                        probs[:, GQ * b:GQ * (b + 1)],
                        start=(b == 0), stop=(b == 3))
                S[g]["zps"] = zps

            def recip_emit(g, S):
                z_f = pz.tile([8, GQ], f32, tag="zf", name="zf")
                z_sb = pz.tile([8, GQ], f32r, tag="z", name="zsb")
                nc.vector.reciprocal_approx_fast(z_f, S[g]["zps"])
                with nc.allow_low_precision(reason="f32r bits are f32"):
                    nc.vector.tensor_copy(z_sb, z_f)
                S[g]["z_sb"] = z_sb

            def rbc_mm_emit(g, S):
                z_r = S[g]["z_sb"]
                rbc = [psB.tile([128, GQ], f32, tag="bb", name="psrbc")
                       for _ in range(2)]
                for c in range(2):
                    nc.tensor.matmul(
                        rbc[c], ebc[:, 128 * c:128 * (c + 1)],
                        z_r, start=True, stop=True)
                S[g]["rbc"] = rbc

            def rbc_copy_emit(g, S):
                rbc_sb = [prb.tile([128, GQ], f32, tag="rbc", name="rbcsb")
                          for _ in range(2)]
                for c in range(2):
                    nc.vector.tensor_copy(rbc_sb[c], S[g]["rbc"][c])
                S[g]["rbc_sb"] = rbc_sb

            def attnv_emit(g, S, v_sb, v_sw):
                l0 = g * GRP
                probs = S[g]["probs"]
                ops = [psB.tile([128, GQ], f32, tag="bb", name="psops")
                       for _ in range(2)]
                for li in range(GRP):
                    l = l0 + li
                    vcol = 256 * (l // 2)
                    for h in range(HEADS):
                        hc, hr = h // 4, h % 4
                        vsrc = v_sb if (l % 2) == hc else v_sw
                        vt = vsrc[64 * hc:64 * (hc + 1),
                                  vcol + 32 * h:vcol + 32 * (h + 1)]
                        pt = probs[64 * hc:64 * (hc + 1),
                                   GQ * hr + 64 * li:GQ * hr + 64 * (li + 1)]
                        nc.tensor.matmul(
                            ops[hc][32 * hr:32 * (hr + 1),
                                    64 * li:64 * (li + 1)],
                            vt, pt, start=True, stop=True,
                            tile_position=(64 * hc, 32 * hr))
                S[g]["ops"] = ops

            def otmul_emit(g, S):
                oT = [po.tile([128, GQ], f32r, tag="oT", name="oT")
                      for _ in range(2)]
                with nc.allow_low_precision(reason="f32r bits are f32"):
                    for c in range(2):
                        nc.vector.tensor_mul(oT[c], S[g]["ops"][c],
                                             S[g]["rbc_sb"][c])
                S[g]["oT"] = oT

            def proj_emit(g, S, bt):
                oT = S[g]["oT"]
                for mc in range(2):
                    ps = psM.tile([128, GQ], f32, tag="mm", name="psproj")
                    for kc in range(2):
                        nc.tensor.matmul(ps, wp[kc][mc], oT[kc],
                                         start=(kc == 0), stop=(kc == 1))
                    osb = pout.tile([128, GQ], f32, tag="out", name="outsb")
                    nc.scalar.activation(osb, ps, AF.Identity,
                                         bias=bp[:, mc:mc + 1], scale=1.0)
                    nc.sync.dma_start(
                        out=out_d[bt, 128 * mc:128 * (mc + 1),
                                  GQ * g:GQ * (g + 1)],
                        in_=osb)
                del S[g]

            def post_head(g, S):
                # PE-early piece of post(g): rbc MMs (before scores of g+2)
                rbc_mm_emit(g, S)

            def post_tail(g, S, v_sb, v_sw, bt):
                rbc_copy_emit(g, S)          # DVE
                attnv_emit(g, S, v_sb, v_sw)  # PE
                otmul_emit(g, S)             # DVE
                proj_emit(g, S, bt)          # PE + scalar + DMA

            # ---------------- main loop ----------------
            xt_next = load_x(0)
            for bt in range(BT_PER_CORE):
                xt = xt_next
                if bt + 1 < BT_PER_CORE:
                    xt_next = load_x(bt + 1)
                qkT, v_sb, v_sw = qkvproj(xt)
                S = {}
                for g in range(NGRP):
                    if g >= 1:
                        z_emit(g - 1, S)          # GPSIMD
                    if g >= 2:
                        post_head(g - 2, S)       # PE (rbc MMs)
                    scores(g, qkT, S)             # PE
                    if g >= 1:
                        recip_emit(g - 1, S)      # DVE (early)
                    exp_emit(g, S)                # scalar
                    ebmul_emit(g, S)              # DVE
                    if g >= 2:
                        post_tail(g - 2, S, v_sb, v_sw, bt)
                # flush
                post_head(NGRP - 2, S)
                post_tail(NGRP - 2, S, v_sb, v_sw, bt)
                z_emit(NGRP - 1, S)
                recip_emit(NGRP - 1, S)
                post_head(NGRP - 1, S)
                post_tail(NGRP - 1, S, v_sb, v_sw, bt)
    nc.compile()
    return nc


def _host_inputs(x, relative_bias, w_qkv, b_qkv, w_proj, b_proj):
    scale = D ** -0.5
    wq = w_qkv[:, :C] * scale          # [256, 256]
    wk = w_qkv[:, C:2 * C]
    wv = w_qkv[:, 2 * C:]
    bqv = b_qkv[:C] * scale            # [256]
    bv = b_qkv[2 * C:]
    wqk_full = np.concatenate([wq, wk], axis=1)        # [256, 512]
    wqk = np.stack([wqk_full[:128], wqk_full[128:]]).astype(np.float32)
    wvs = np.stack([wv[:128], wv[128:]]).astype(np.float32)
    wp = np.zeros((2, 2, 128, 128), np.float32)
    for kc in range(2):
        for mc in range(2):
            wp[kc, mc] = w_proj[128 * kc:128 * (kc + 1),
                                128 * mc:128 * (mc + 1)]
    bq = np.stack([bqv[:128], bqv[128:]], axis=1).astype(np.float32)  # [128,2]
    bpv = bv @ w_proj + b_proj                                       # [256]
    bp = np.stack([bpv[:128], bpv[128:]], axis=1).astype(np.float32)
    # exp(bias) tiles, matching probs layout: bank hr, partition half hh =
    # head h = hr + 4*hh; [k, q] = bias[h].T; tiled 8 lines along free.
    eb = np.zeros((128, 2048), np.float32)
    for hr in range(4):
        for hh in range(2):
            h = hr + 4 * hh
            ebT = np.exp(relative_bias[h].T.astype(np.float64)).astype(np.float32)
            eb[64 * hh:64 * (hh + 1), 512 * hr:512 * (hr + 1)] = \
                np.tile(ebT, (1, 8))
    ez = np.zeros((128, 32), np.float32)
    for b in range(4):
        ez[0:64, 8 * b + 2 * b] = 1        # head b    -> z row 2b
        ez[64:128, 8 * b + 2 * b + 1] = 1  # head b+4  -> z row 2b+1
    ebc = np.zeros((8, 256), np.float32)
    for c in range(2):
        for hr in range(4):
            h = 4 * c + hr
            zrow = 2 * (h % 4) + (h // 4)
            ebc[zrow, 128 * c + 32 * hr:128 * c + 32 * (hr + 1)] = 1.0
    st = np.zeros((128, ST_COLS), np.float32)
    st[:, 0:512] = wqk[0]
    st[:, 512:1024] = wqk[1]
    st[:, 1024:1280] = wvs[0]
    st[:, 1280:1536] = wvs[1]
    for i in range(2):
        for j in range(2):
            st[:, 1536 + 256 * i + 128 * j:1536 + 256 * i + 128 * (j + 1)] = wp[i, j]
    st[:, 2048:4096] = eb
    st[:, 4096:4098] = bq
    st[:, 4098:4100] = bp
    st[0:8, 4100:4356] = ebc
    st[:, 4356:4388] = ez
    return dict(statics=st)


def kernel(x, relative_bias, w_qkv, b_qkv, w_proj, b_proj):
    import sys
    if '/opt/trn_rl_repo' not in sys.path:
        sys.path.insert(0, '/opt/trn_rl_repo')
    from concourse.bass_utils import run_bass_kernel_spmd

    x = np.asarray(x, np.float32)
    const = _host_inputs(np.asarray(x, np.float32),
                         np.asarray(relative_bias, np.float32),
                         np.asarray(w_qkv, np.float32),
                         np.asarray(b_qkv, np.float32),
                         np.asarray(w_proj, np.float32),
                         np.asarray(b_proj, np.float32))
    # x [B,T,C,H,W] -> [64, 256, 4096]
    xr = np.ascontiguousarray(x.reshape(NBT, C, HW))
    nc = _build_bass()
    in_maps = []
    for c in range(NCORES):
        m = dict(const)
        m["x"] = np.ascontiguousarray(xr[c * BT_PER_CORE:(c + 1) * BT_PER_CORE])
        in_maps.append(m)
    res = run_bass_kernel_spmd(nc, in_maps, list(range(NCORES)))
    globals()["LAST_RESULT"] = res
    outs = res.results
    out = np.concatenate([o["out"].reshape(BT_PER_CORE, C, HW) for o in outs],
                         axis=0)
    return out.reshape(B, T, C, H, W).astype(np.float32)


# revision 29
# speedup vs baseline: 1.2262x; 1.1008x over previous
"""AxialAttention TRN2 Bass kernel (pipelined).

Shapes (hardcoded): x [B=4,T=16,C=256,H=64,W=64] fp32.
N = B*T*H = 4096 lines of [L=64, C=256]; heads=8, d=32.
Sharding: 64 (b,t) blocks -> 8 per core across 8 cores.

Per-core dataflow, per (b,t) block (xT = x[b,t] viewed [C=256, HW=4096]):
  qkT  = w_qk^T @ xT          (fp32r MMs, N=512)   q pre-scaled by 1/sqrt(d)
  v    = xT^T @ w_v           (row-major v)
  per 8-line group g, per psum bank hr (heads hr, hr+4 stacked):
    scoresT = kT^T @ qT        (bf16 packed MMs, start=True; NO bias seed)
    probs0  = exp(scoresT)     (ACT -> bf16 sbuf)
    probs   = probs0 * exp(bias)  (DVE bf16 mul; softmax bias via factoring)
    Z       = Ez^T @ probs     (column sums, psum [8, 512])
    rz      = recip_approx_fast(Z)   (DVE custom op)
    rbc     = Ebc^T @ rz       (broadcast rz over d=32 partitions)
    oT      = v^T @ probs      (bf16 packed MMs)
    oT_sb   = oT * rbc
    outT    = w_proj^T @ oT_sb + b'
Software pipeline (emission order per iteration g):
  PE:     Z(g-1), rbc(g-2), scores(g), attnv(g-2), proj(g-2)
  Scalar: exp(g), proj-evac(g-2)
  DVE:    recip(g-1), rbc-copy(g-2), oT-mul(g-2), ebmul(g)
PSUM: psA=4 (scores), psB=2 (Z/rbc/ops), psM=2 (qkv+proj MMs).
b_k dropped (softmax shift-invariant); b_q folded into q evacuation bias;
b_v folded into b' = b_v @ w_proj + b_proj.
"""

import numpy as np

B, T, C, H, W = 4, 16, 256, 64, 64
HEADS, D = 8, 32
NBT = B * T            # 64 (b,t) blocks
NCORES = 8
BT_PER_CORE = NBT // NCORES  # 8
HW = H * W             # 4096 positions per block
L = W                  # 64
GRP = 8                # lines per attention group
NGRP = H // GRP        # 8 groups per block
GQ = GRP * L           # 512 free columns per group

ST_COLS = 4388


def _build_bass():
    import concourse.bacc as bacc
    import concourse.mybir as mybir
    from concourse.tile import TileContext

    f32 = mybir.dt.float32
    f32r = mybir.dt.float32r
    bf16 = mybir.dt.bfloat16
    AF = mybir.ActivationFunctionType

    nc = bacc.Bacc("TRN2", target_bir_lowering=False, debug=False,
                   num_devices=NCORES)

    x_d = nc.dram_tensor("x", [BT_PER_CORE, C, HW], f32r, kind="ExternalInput").ap()
    st_d = nc.dram_tensor("statics", [128, ST_COLS], f32r, kind="ExternalInput").ap()
    out_d = nc.dram_tensor("out", [BT_PER_CORE, C, HW], f32, kind="ExternalOutput").ap()

    with TileContext(nc) as tc:
        with (
            tc.tile_pool(name="static", bufs=1) as stat,
            tc.tile_pool(name="xt", bufs=2) as pxt,
            tc.tile_pool(name="qk", bufs=4) as pqk,
            tc.tile_pool(name="vsb", bufs=2) as pv,
            tc.tile_pool(name="praw", bufs=1) as praw,
            tc.tile_pool(name="probs", bufs=3) as ppr,
            tc.tile_pool(name="zsb", bufs=2) as pz,
            tc.tile_pool(name="rbc", bufs=2) as prb,
            tc.tile_pool(name="osb", bufs=2) as po,
            tc.tile_pool(name="outsb", bufs=2) as pout,
            tc.tile_pool(name="psA", bufs=4, space="PSUM") as psA,
            tc.tile_pool(name="psB", bufs=2, space="PSUM") as psB,
            tc.tile_pool(name="psM", bufs=2, space="PSUM") as psM,
        ):
            # ---- static loads: one packed DMA ----
            st = stat.tile([128, ST_COLS], f32r, tag="st", name="statics_sb")
            nc.sync.dma_start(out=st, in_=st_d)
            wqk = [st[:, 512 * i:512 * (i + 1)] for i in range(2)]
            wv = [st[:, 1024 + 256 * i:1024 + 256 * (i + 1)] for i in range(2)]
            wp = [[st[:, 1536 + 256 * i + 128 * j:1536 + 256 * i + 128 * (j + 1)]
                   for j in range(2)] for i in range(2)]
            eb_f32 = st[:, 2048:4096]
            bq = st[:, 4096:4098].bitcast(f32)
            bp = st[:, 4098:4100].bitcast(f32)
            ebc = st[0:8, 4100:4356]
            ez = stat.tile([128, 32], bf16, tag="ez", name="ez")
            eb = stat.tile([128, 2048], bf16, tag="eb", name="eb")
            with nc.allow_low_precision(reason="exact constants / bias factor"):
                nc.vector.tensor_copy(ez, st[:, 4356:4388])
                nc.vector.tensor_copy(eb, eb_f32)


            # ---------------- emission helpers ----------------
            def load_x(bt):
                tiles = [pxt.tile([128, HW], f32r, tag="xt", name="xt")
                         for _ in range(2)]
                for kc in range(2):
                    nc.sync.dma_start(
                        out=tiles[kc][:, :2048],
                        in_=x_d[bt, 128 * kc:128 * (kc + 1), :2048])
                    nc.sync.dma_start(
                        out=tiles[kc][:, 2048:],
                        in_=x_d[bt, 128 * kc:128 * (kc + 1), 2048:])
                return tiles

            def qkvproj(xt):
                qkT = [pqk.tile([128, HW], bf16, tag="qkT", name="qkT")
                       for _ in range(4)]
                # qk projection: qkT [512, 4096] -> bf16 sbuf
                for mc in range(4):
                    for nn in range(8):
                        ps = psM.tile([128, 512], f32, tag="mm", name="psmm")
                        for kc in range(2):
                            nc.tensor.matmul(
                                ps, wqk[kc][:, 128 * mc:128 * (mc + 1)],
                                xt[kc][:, 512 * nn:512 * (nn + 1)],
                                start=(kc == 0), stop=(kc == 1))
                        dst = qkT[mc][:, 512 * nn:512 * (nn + 1)]
                        if mc < 2:  # q: fold b_q (pre-scaled) per-partition
                            nc.scalar.activation(dst, ps, AF.Identity,
                                                 bias=bq[:, mc:mc + 1], scale=1.0)
                        else:       # k: plain copy (b_k dropped)
                            nc.vector.tensor_copy(dst, ps)
                # v projection (row-major): v_sb [128 = 2 lines, 32 chunks * 256]
                v_sb = pv.tile([128, 8192], bf16, tag="vsb", name="vsb")
                v_sw = pv.tile([128, 8192], bf16, tag="vsw", name="vsw")
                for pc in range(32):
                    ps = psM.tile([128, 256], f32, tag="mm", name="psmmv")
                    for kc in range(2):
                        nc.tensor.matmul(
                            ps, xt[kc][:, 128 * pc:128 * (pc + 1)],
                            wv[kc], start=(kc == 0), stop=(kc == 1))
                    if pc % 2 == 0:
                        nc.scalar.copy(v_sb[:, 256 * pc:256 * (pc + 1)], ps)
                    else:
                        nc.vector.tensor_copy(v_sb[:, 256 * pc:256 * (pc + 1)], ps)
                    if pc % 8 == 7:  # pipelined partition-swap of finished span
                        c0, c1 = 256 * (pc - 7), 256 * (pc + 1)
                        nc.sync.dma_start(out=v_sw[0:64, c0:c1],
                                          in_=v_sb[64:128, c0:c1])
                        nc.sync.dma_start(out=v_sw[64:128, c0:c1],
                                          in_=v_sb[0:64, c0:c1])
                return qkT, v_sb, v_sw

            def scores(g, qkT, S):
                l0 = g * GRP
                sps = [psA.tile([128, GQ], f32, tag="att", name="psatt")
                       for _ in range(4)]
                for li in range(GRP):
                    l = l0 + li
                    for h in range(HEADS):
                        hc, hr = h // 4, h % 4
                        kt = qkT[2 + hc][32 * hr:32 * (hr + 1),
                                         64 * l:64 * (l + 1)]
                        qt = qkT[hc][32 * hr:32 * (hr + 1),
                                     64 * l:64 * (l + 1)]
                        nc.tensor.matmul(
                            sps[hr][64 * hc:64 * (hc + 1),
                                    64 * li:64 * (li + 1)],
                            kt, qt, start=True, stop=True,
                            tile_position=(32 * hr, 64 * hc))
                S[g] = {"sps": sps}

            def exp_emit(g, S):
                sps = S[g]["sps"]
                probs0 = praw.tile([128, 4 * GQ], bf16, tag="praw", name="praw")
                for b in range(4):
                    nc.scalar.activation(
                        probs0[:, GQ * b:GQ * (b + 1)], sps[b], AF.Exp,
                        scale=1.0)
                S[g]["probs0"] = probs0

            def ebmul_emit(g, S):
                probs0 = S[g]["probs0"]
                probs = ppr.tile([128, 4 * GQ], bf16, tag="probs", name="probs")
                with nc.allow_low_precision(reason="softmax weights bf16"):
                    for b in range(4):
                        nc.vector.tensor_mul(
                            probs[:, GQ * b:GQ * (b + 1)],
                            probs0[:, GQ * b:GQ * (b + 1)],
                            eb[:, GQ * b:GQ * (b + 1)])
                S[g]["probs"] = probs

            def z_emit(g, S):
                probs = S[g]["probs"]
                zps = psB.tile([8, GQ], f32, tag="bb", name="psz")
                for b in range(4):
                    nc.tensor.matmul(
                        zps, ez[:, 8 * b:8 * (b + 1)],
                        probs[:, GQ * b:GQ * (b + 1)],
                        start=(b == 0), stop=(b == 3))
                S[g]["zps"] = zps

            def recip_emit(g, S):
                z_f = pz.tile([8, GQ], f32, tag="zf", name="zf")
                z_sb = pz.tile([8, GQ], f32r, tag="z", name="zsb")
                nc.vector.reciprocal_approx_fast(z_f, S[g]["zps"])
                with nc.allow_low_precision(reason="f32r bits are f32"):
                    nc.vector.tensor_copy(z_sb, z_f)
                S[g]["z_sb"] = z_sb

            def rbc_mm_emit(g, S):
                z_r = S[g]["z_sb"]
                rbc = [psB.tile([128, GQ], f32, tag="bb", name="psrbc")
                       for _ in range(2)]
                for c in range(2):
                    nc.tensor.matmul(
                        rbc[c], ebc[:, 128 * c:128 * (c + 1)],
                        z_r, start=True, stop=True)
                S[g]["rbc"] = rbc

            def rbc_copy_emit(g, S):
                rbc_sb = [prb.tile([128, GQ], f32, tag="rbc", name="rbcsb")
                          for _ in range(2)]
                for c in range(2):
                    nc.vector.tensor_copy(rbc_sb[c], S[g]["rbc"][c])
                S[g]["rbc_sb"] = rbc_sb

            def attnv_emit(g, S, v_sb, v_sw):
                l0 = g * GRP
                probs = S[g]["probs"]
                ops = [psB.tile([128, GQ], f32, tag="bb", name="psops")
                       for _ in range(2)]
                for li in range(GRP):
                    l = l0 + li
                    vcol = 256 * (l // 2)
                    for h in range(HEADS):
                        hc, hr = h // 4, h % 4
                        vsrc = v_sb if (l % 2) == hc else v_sw
                        vt = vsrc[64 * hc:64 * (hc + 1),
                                  vcol + 32 * h:vcol + 32 * (h + 1)]
                        pt = probs[64 * hc:64 * (hc + 1),
                                   GQ * hr + 64 * li:GQ * hr + 64 * (li + 1)]
                        nc.tensor.matmul(
                            ops[hc][32 * hr:32 * (hr + 1),
                                    64 * li:64 * (li + 1)],
                            vt, pt, start=True, stop=True,
                            tile_position=(64 * hc, 32 * hr))
                S[g]["ops"] = ops

            def otmul_emit(g, S):
                oT = [po.tile([128, GQ], f32r, tag="oT", name="oT")
                      for _ in range(2)]
                with nc.allow_low_precision(reason="f32r bits are f32"):
                    for c in range(2):
                        nc.vector.tensor_mul(oT[c], S[g]["ops"][c],
                                             S[g]["rbc_sb"][c])
                S[g]["oT"] = oT

            def proj_emit(g, S, bt):
                oT = S[g]["oT"]
                for mc in range(2):
                    ps = psM.tile([128, GQ], f32, tag="mm", name="psproj")
                    for kc in range(2):
                        nc.tensor.matmul(ps, wp[kc][mc], oT[kc],
                                         start=(kc == 0), stop=(kc == 1))
                    osb = pout.tile([128, GQ], f32, tag="out", name="outsb")
                    nc.scalar.activation(osb, ps, AF.Identity,
                                         bias=bp[:, mc:mc + 1], scale=1.0)
                    nc.sync.dma_start(
                        out=out_d[bt, 128 * mc:128 * (mc + 1),
                                  GQ * g:GQ * (g + 1)],
                        in_=osb)
                del S[g]

            def post_head(g, S):
                # PE-early piece of post(g): rbc MMs (before scores of g+2)
                rbc_mm_emit(g, S)

            def post_tail(g, S, v_sb, v_sw, bt):
                rbc_copy_emit(g, S)          # DVE
                attnv_emit(g, S, v_sb, v_sw)  # PE
                otmul_emit(g, S)             # DVE
                proj_emit(g, S, bt)          # PE + scalar + DMA

            # ---------------- main loop ----------------
            xt_next = load_x(0)
            for bt in range(BT_PER_CORE):
                xt = xt_next
                if bt + 1 < BT_PER_CORE:
                    xt_next = load_x(bt + 1)
                qkT, v_sb, v_sw = qkvproj(xt)
                S = {}
                for g in range(NGRP):
                    if g >= 1:
                        z_emit(g - 1, S)          # GPSIMD
                    if g >= 2:
                        post_head(g - 2, S)       # PE (rbc MMs)
                    scores(g, qkT, S)             # PE
                    if g >= 1:
                        recip_emit(g - 1, S)      # DVE (early)
                    exp_emit(g, S)                # scalar
                    ebmul_emit(g, S)              # DVE
                    if g >= 2:
                        post_tail(g - 2, S, v_sb, v_sw, bt)
                # flush: Z/recip(7) early so recip isn't queued behind the
                # oT-muls on DVE (which would stall rbc(7) on the PE)
                post_head(NGRP - 2, S)
                z_emit(NGRP - 1, S)
                recip_emit(NGRP - 1, S)
                post_tail(NGRP - 2, S, v_sb, v_sw, bt)
                post_head(NGRP - 1, S)
                post_tail(NGRP - 1, S, v_sb, v_sw, bt)
    nc.compile()
    return nc


def _host_inputs(x, relative_bias, w_qkv, b_qkv, w_proj, b_proj):
    scale = D ** -0.5
    wq = w_qkv[:, :C] * scale          # [256, 256]
    wk = w_qkv[:, C:2 * C]
    wv = w_qkv[:, 2 * C:]
    bqv = b_qkv[:C] * scale            # [256]
    bv = b_qkv[2 * C:]
    wqk_full = np.concatenate([wq, wk], axis=1)        # [256, 512]
    wqk = np.stack([wqk_full[:128], wqk_full[128:]]).astype(np.float32)
    wvs = np.stack([wv[:128], wv[128:]]).astype(np.float32)
    wp = np.zeros((2, 2, 128, 128), np.float32)
    for kc in range(2):
        for mc in range(2):
            wp[kc, mc] = w_proj[128 * kc:128 * (kc + 1),
                                128 * mc:128 * (mc + 1)]
    bq = np.stack([bqv[:128], bqv[128:]], axis=1).astype(np.float32)  # [128,2]
    bpv = bv @ w_proj + b_proj                                       # [256]
    bp = np.stack([bpv[:128], bpv[128:]], axis=1).astype(np.float32)
    # exp(bias) tiles, matching probs layout: bank hr, partition half hh =
    # head h = hr + 4*hh; [k, q] = bias[h].T; tiled 8 lines along free.
    eb = np.zeros((128, 2048), np.float32)
    for hr in range(4):
        for hh in range(2):
            h = hr + 4 * hh
            ebT = np.exp(relative_bias[h].T.astype(np.float64)).astype(np.float32)
            eb[64 * hh:64 * (hh + 1), 512 * hr:512 * (hr + 1)] = \
                np.tile(ebT, (1, 8))
    ez = np.zeros((128, 32), np.float32)
    for b in range(4):
        ez[0:64, 8 * b + 2 * b] = 1        # head b    -> z row 2b
        ez[64:128, 8 * b + 2 * b + 1] = 1  # head b+4  -> z row 2b+1
    ebc = np.zeros((8, 256), np.float32)
    for c in range(2):
        for hr in range(4):
            h = 4 * c + hr
            zrow = 2 * (h % 4) + (h // 4)
            ebc[zrow, 128 * c + 32 * hr:128 * c + 32 * (hr + 1)] = 1.0
    st = np.zeros((128, ST_COLS), np.float32)
    st[:, 0:512] = wqk[0]
    st[:, 512:1024] = wqk[1]
    st[:, 1024:1280] = wvs[0]
    st[:, 1280:1536] = wvs[1]
    for i in range(2):
        for j in range(2):
            st[:, 1536 + 256 * i + 128 * j:1536 + 256 * i + 128 * (j + 1)] = wp[i, j]
    st[:, 2048:4096] = eb
    st[:, 4096:4098] = bq
    st[:, 4098:4100] = bp
    st[0:8, 4100:4356] = ebc
    st[:, 4356:4388] = ez
    return dict(statics=st)


def kernel(x, relative_bias, w_qkv, b_qkv, w_proj, b_proj):
    import sys
    if '/opt/trn_rl_repo' not in sys.path:
        sys.path.insert(0, '/opt/trn_rl_repo')
    from concourse.bass_utils import run_bass_kernel_spmd

    x = np.asarray(x, np.float32)
    const = _host_inputs(np.asarray(x, np.float32),
                         np.asarray(relative_bias, np.float32),
                         np.asarray(w_qkv, np.float32),
                         np.asarray(b_qkv, np.float32),
                         np.asarray(w_proj, np.float32),
                         np.asarray(b_proj, np.float32))
    # x [B,T,C,H,W] -> [64, 256, 4096]
    xr = np.ascontiguousarray(x.reshape(NBT, C, HW))
    nc = _build_bass()
    in_maps = []
    for c in range(NCORES):
        m = dict(const)
        m["x"] = np.ascontiguousarray(xr[c * BT_PER_CORE:(c + 1) * BT_PER_CORE])
        in_maps.append(m)
    res = run_bass_kernel_spmd(nc, in_maps, list(range(NCORES)))
    globals()["LAST_RESULT"] = res
    outs = res.results
    out = np.concatenate([o["out"].reshape(BT_PER_CORE, C, HW) for o in outs],
                         axis=0)
    return out.reshape(B, T, C, H, W).astype(np.float32)


# revision 30
# speedup vs baseline: 1.2342x; 1.0065x over previous
"""AxialAttention TRN2 Bass kernel (pipelined).

Shapes (hardcoded): x [B=4,T=16,C=256,H=64,W=64] fp32.
N = B*T*H = 4096 lines of [L=64, C=256]; heads=8, d=32.
Sharding: 64 (b,t) blocks -> 8 per core across 8 cores.

Per-core dataflow, per (b,t) block (xT = x[b,t] viewed [C=256, HW=4096]):
  qkT  = w_qk^T @ xT          (fp32r MMs, N=512)   q pre-scaled by 1/sqrt(d)
  v    = xT^T @ w_v           (row-major v)
  per 8-line group g, per psum bank hr (heads hr, hr+4 stacked):
    scoresT = kT^T @ qT        (bf16 packed MMs, start=True; NO bias seed)
    probs0  = exp(scoresT)     (ACT -> bf16 sbuf)
    probs   = probs0 * exp(bias)  (DVE bf16 mul; softmax bias via factoring)
    Z       = Ez^T @ probs     (column sums, psum [8, 512])
    rz      = recip_approx_fast(Z)   (DVE custom op)
    rbc     = Ebc^T @ rz       (broadcast rz over d=32 partitions)
    oT      = v^T @ probs      (bf16 packed MMs)
    oT_sb   = oT * rbc
    outT    = w_proj^T @ oT_sb + b'
Software pipeline (emission order per iteration g):
  PE:     Z(g-1), rbc(g-2), scores(g), attnv(g-2), proj(g-2)
  Scalar: exp(g), proj-evac(g-2)
  DVE:    recip(g-1), rbc-copy(g-2), oT-mul(g-2), ebmul(g)
PSUM: psA=4 (scores), psB=2 (Z/rbc/ops), psM=2 (qkv+proj MMs).
b_k dropped (softmax shift-invariant); b_q folded into q evacuation bias;
b_v folded into b' = b_v @ w_proj + b_proj.
"""

import numpy as np

B, T, C, H, W = 4, 16, 256, 64, 64
HEADS, D = 8, 32
NBT = B * T            # 64 (b,t) blocks
NCORES = 8
BT_PER_CORE = NBT // NCORES  # 8
HW = H * W             # 4096 positions per block
L = W                  # 64
GRP = 8                # lines per attention group
NGRP = H // GRP        # 8 groups per block
GQ = GRP * L           # 512 free columns per group

ST_COLS = 4388


def _build_bass():
    import concourse.bacc as bacc
    import concourse.mybir as mybir
    from concourse.tile import TileContext

    f32 = mybir.dt.float32
    f32r = mybir.dt.float32r
    bf16 = mybir.dt.bfloat16
    AF = mybir.ActivationFunctionType

    nc = bacc.Bacc("TRN2", target_bir_lowering=False, debug=False,
                   num_devices=NCORES)

    x_d = nc.dram_tensor("x", [BT_PER_CORE, C, HW], f32r, kind="ExternalInput").ap()
    st_d = nc.dram_tensor("statics", [128, ST_COLS], f32r, kind="ExternalInput").ap()
    out_d = nc.dram_tensor("out", [BT_PER_CORE, C, HW], f32, kind="ExternalOutput").ap()

    with TileContext(nc) as tc:
        with (
            tc.tile_pool(name="static", bufs=1) as stat,
            tc.tile_pool(name="xt", bufs=2) as pxt,
            tc.tile_pool(name="qk", bufs=4) as pqk,
            tc.tile_pool(name="vsb", bufs=2) as pv,
            tc.tile_pool(name="praw", bufs=2) as praw,
            tc.tile_pool(name="probs", bufs=3) as ppr,
            tc.tile_pool(name="zsb", bufs=2) as pz,
            tc.tile_pool(name="rbc", bufs=3) as prb,
            tc.tile_pool(name="osb", bufs=3) as po,
            tc.tile_pool(name="outsb", bufs=3) as pout,
            tc.tile_pool(name="psA", bufs=4, space="PSUM") as psA,
            tc.tile_pool(name="psB", bufs=2, space="PSUM") as psB,
            tc.tile_pool(name="psM", bufs=2, space="PSUM") as psM,
        ):
            # ---- static loads: one packed DMA ----
            st = stat.tile([128, ST_COLS], f32r, tag="st", name="statics_sb")
            nc.sync.dma_start(out=st, in_=st_d)
            wqk = [st[:, 512 * i:512 * (i + 1)] for i in range(2)]
            wv = [st[:, 1024 + 256 * i:1024 + 256 * (i + 1)] for i in range(2)]
            wp = [[st[:, 1536 + 256 * i + 128 * j:1536 + 256 * i + 128 * (j + 1)]
                   for j in range(2)] for i in range(2)]
            eb_f32 = st[:, 2048:4096]
            bq = st[:, 4096:4098].bitcast(f32)
            bp = st[:, 4098:4100].bitcast(f32)
            ebc = st[0:8, 4100:4356]
            ez = stat.tile([128, 32], bf16, tag="ez", name="ez")
            eb = stat.tile([128, 2048], bf16, tag="eb", name="eb")
            with nc.allow_low_precision(reason="exact constants / bias factor"):
                nc.vector.tensor_copy(ez, st[:, 4356:4388])
                nc.vector.tensor_copy(eb, eb_f32)


            # ---------------- emission helpers ----------------
            def load_x(bt):
                tiles = [pxt.tile([128, HW], f32r, tag="xt", name="xt")
                         for _ in range(2)]
                for kc in range(2):
                    nc.sync.dma_start(
                        out=tiles[kc][:, :2048],
                        in_=x_d[bt, 128 * kc:128 * (kc + 1), :2048])
                    nc.sync.dma_start(
                        out=tiles[kc][:, 2048:],
                        in_=x_d[bt, 128 * kc:128 * (kc + 1), 2048:])
                return tiles

            def qkvproj(xt):
                qkT = [pqk.tile([128, HW], bf16, tag="qkT", name="qkT")
                       for _ in range(4)]
                # qk projection: qkT [512, 4096] -> bf16 sbuf
                for mc in range(4):
                    for nn in range(8):
                        ps = psM.tile([128, 512], f32, tag="mm", name="psmm")
                        for kc in range(2):
                            nc.tensor.matmul(
                                ps, wqk[kc][:, 128 * mc:128 * (mc + 1)],
                                xt[kc][:, 512 * nn:512 * (nn + 1)],
                                start=(kc == 0), stop=(kc == 1))
                        dst = qkT[mc][:, 512 * nn:512 * (nn + 1)]
                        if mc < 2:  # q: fold b_q (pre-scaled) per-partition
                            nc.scalar.activation(dst, ps, AF.Identity,
                                                 bias=bq[:, mc:mc + 1], scale=1.0)
                        else:       # k: plain copy (b_k dropped)
                            nc.vector.tensor_copy(dst, ps)
                # v projection (row-major): v_sb [128 = 2 lines, 32 chunks * 256]
                v_sb = pv.tile([128, 8192], bf16, tag="vsb", name="vsb")
                v_sw = pv.tile([128, 8192], bf16, tag="vsw", name="vsw")
                for pc in range(32):
                    ps = psM.tile([128, 256], f32, tag="mm", name="psmmv")
                    for kc in range(2):
                        nc.tensor.matmul(
                            ps, xt[kc][:, 128 * pc:128 * (pc + 1)],
                            wv[kc], start=(kc == 0), stop=(kc == 1))
                    if pc % 2 == 0:
                        nc.scalar.copy(v_sb[:, 256 * pc:256 * (pc + 1)], ps)
                    else:
                        nc.vector.tensor_copy(v_sb[:, 256 * pc:256 * (pc + 1)], ps)
                    if pc % 8 == 7:  # pipelined partition-swap of finished span
                        c0, c1 = 256 * (pc - 7), 256 * (pc + 1)
                        nc.sync.dma_start(out=v_sw[0:64, c0:c1],
                                          in_=v_sb[64:128, c0:c1])
                        nc.sync.dma_start(out=v_sw[64:128, c0:c1],
                                          in_=v_sb[0:64, c0:c1])
                return qkT, v_sb, v_sw

            def scores(g, qkT, S):
                l0 = g * GRP
                sps = [psA.tile([128, GQ], f32, tag="att", name="psatt")
                       for _ in range(4)]
                for li in range(GRP):
                    l = l0 + li
                    for h in range(HEADS):
                        hc, hr = h // 4, h % 4
                        kt = qkT[2 + hc][32 * hr:32 * (hr + 1),
                                         64 * l:64 * (l + 1)]
                        qt = qkT[hc][32 * hr:32 * (hr + 1),
                                     64 * l:64 * (l + 1)]
                        nc.tensor.matmul(
                            sps[hr][64 * hc:64 * (hc + 1),
                                    64 * li:64 * (li + 1)],
                            kt, qt, start=True, stop=True,
                            tile_position=(32 * hr, 64 * hc))
                S[g] = {"sps": sps}

            def exp_emit(g, S):
                sps = S[g]["sps"]
                probs0 = praw.tile([128, 4 * GQ], bf16, tag="praw", name="praw")
                for b in range(4):
                    nc.scalar.activation(
                        probs0[:, GQ * b:GQ * (b + 1)], sps[b], AF.Exp,
                        scale=1.0)
                S[g]["probs0"] = probs0

            def ebmul_emit(g, S):
                probs0 = S[g]["probs0"]
                probs = ppr.tile([128, 4 * GQ], bf16, tag="probs", name="probs")
                with nc.allow_low_precision(reason="softmax weights bf16"):
                    for b in range(4):
                        nc.vector.tensor_mul(
                            probs[:, GQ * b:GQ * (b + 1)],
                            probs0[:, GQ * b:GQ * (b + 1)],
                            eb[:, GQ * b:GQ * (b + 1)])
                S[g]["probs"] = probs

            def z_emit(g, S):
                probs = S[g]["probs"]
                zps = psB.tile([8, GQ], f32, tag="bb", name="psz")
                for b in range(4):
                    nc.tensor.matmul(
                        zps, ez[:, 8 * b:8 * (b + 1)],
                        probs[:, GQ * b:GQ * (b + 1)],
                        start=(b == 0), stop=(b == 3))
                S[g]["zps"] = zps

            def recip_emit(g, S):
                z_f = pz.tile([8, GQ], f32, tag="zf", name="zf")
                z_sb = pz.tile([8, GQ], f32r, tag="z", name="zsb")
                nc.vector.reciprocal_approx_fast(z_f, S[g]["zps"])
                with nc.allow_low_precision(reason="f32r bits are f32"):
                    nc.vector.tensor_copy(z_sb, z_f)
                S[g]["z_sb"] = z_sb

            def rbc_mm_emit(g, S):
                z_r = S[g]["z_sb"]
                rbc = [psB.tile([128, GQ], f32, tag="bb", name="psrbc")
                       for _ in range(2)]
                for c in range(2):
                    nc.tensor.matmul(
                        rbc[c], ebc[:, 128 * c:128 * (c + 1)],
                        z_r, start=True, stop=True)
                S[g]["rbc"] = rbc

            def rbc_copy_emit(g, S):
                rbc_sb = [prb.tile([128, GQ], f32, tag="rbc", name="rbcsb")
                          for _ in range(2)]
                for c in range(2):
                    nc.vector.tensor_copy(rbc_sb[c], S[g]["rbc"][c])
                S[g]["rbc_sb"] = rbc_sb

            def attnv_emit(g, S, v_sb, v_sw):
                l0 = g * GRP
                probs = S[g]["probs"]
                ops = [psB.tile([128, GQ], f32, tag="bb", name="psops")
                       for _ in range(2)]
                for li in range(GRP):
                    l = l0 + li
                    vcol = 256 * (l // 2)
                    for h in range(HEADS):
                        hc, hr = h // 4, h % 4
                        vsrc = v_sb if (l % 2) == hc else v_sw
                        vt = vsrc[64 * hc:64 * (hc + 1),
                                  vcol + 32 * h:vcol + 32 * (h + 1)]
                        pt = probs[64 * hc:64 * (hc + 1),
                                   GQ * hr + 64 * li:GQ * hr + 64 * (li + 1)]
                        nc.tensor.matmul(
                            ops[hc][32 * hr:32 * (hr + 1),
                                    64 * li:64 * (li + 1)],
                            vt, pt, start=True, stop=True,
                            tile_position=(64 * hc, 32 * hr))
                S[g]["ops"] = ops

            def otmul_emit(g, S):
                oT = [po.tile([128, GQ], f32r, tag="oT", name="oT")
                      for _ in range(2)]
                with nc.allow_low_precision(reason="f32r bits are f32"):
                    for c in range(2):
                        nc.vector.tensor_mul(oT[c], S[g]["ops"][c],
                                             S[g]["rbc_sb"][c])
                S[g]["oT"] = oT

            def proj_emit(g, S, bt):
                oT = S[g]["oT"]
                for mc in range(2):
                    ps = psM.tile([128, GQ], f32, tag="mm", name="psproj")
                    for kc in range(2):
                        nc.tensor.matmul(ps, wp[kc][mc], oT[kc],
                                         start=(kc == 0), stop=(kc == 1))
                    osb = pout.tile([128, GQ], f32, tag="out", name="outsb")
                    nc.scalar.activation(osb, ps, AF.Identity,
                                         bias=bp[:, mc:mc + 1], scale=1.0)
                    nc.sync.dma_start(
                        out=out_d[bt, 128 * mc:128 * (mc + 1),
                                  GQ * g:GQ * (g + 1)],
                        in_=osb)
                del S[g]

            def post_head(g, S):
                # PE-early piece of post(g): rbc MMs (before scores of g+2)
                rbc_mm_emit(g, S)

            def post_tail(g, S, v_sb, v_sw, bt):
                rbc_copy_emit(g, S)          # DVE
                attnv_emit(g, S, v_sb, v_sw)  # PE
                otmul_emit(g, S)             # DVE
                proj_emit(g, S, bt)          # PE + scalar + DMA

            # ---------------- main loop ----------------
            xt_next = load_x(0)
            for bt in range(BT_PER_CORE):
                xt = xt_next
                if bt + 1 < BT_PER_CORE:
                    xt_next = load_x(bt + 1)
                qkT, v_sb, v_sw = qkvproj(xt)
                S = {}
                for g in range(NGRP):
                    if g >= 1:
                        z_emit(g - 1, S)          # GPSIMD
                    if g >= 2:
                        post_head(g - 2, S)       # PE (rbc MMs)
                    scores(g, qkT, S)             # PE
                    if g >= 1:
                        recip_emit(g - 1, S)      # DVE (early)
                    exp_emit(g, S)                # scalar
                    ebmul_emit(g, S)              # DVE
                    if g >= 2:
                        post_tail(g - 2, S, v_sb, v_sw, bt)
                # flush: Z/recip(7) early so recip isn't queued behind the
                # oT-muls on DVE (which would stall rbc(7) on the PE)
                post_head(NGRP - 2, S)
                z_emit(NGRP - 1, S)
                recip_emit(NGRP - 1, S)
                post_tail(NGRP - 2, S, v_sb, v_sw, bt)
                post_head(NGRP - 1, S)
                post_tail(NGRP - 1, S, v_sb, v_sw, bt)
    nc.compile()
    return nc


def _host_inputs(x, relative_bias, w_qkv, b_qkv, w_proj, b_proj):
    scale = D ** -0.5
    wq = w_qkv[:, :C] * scale          # [256, 256]
    wk = w_qkv[:, C:2 * C]
    wv = w_qkv[:, 2 * C:]
    bqv = b_qkv[:C] * scale            # [256]
    bv = b_qkv[2 * C:]
    wqk_full = np.concatenate([wq, wk], axis=1)        # [256, 512]
    wqk = np.stack([wqk_full[:128], wqk_full[128:]]).astype(np.float32)
    wvs = np.stack([wv[:128], wv[128:]]).astype(np.float32)
    wp = np.zeros((2, 2, 128, 128), np.float32)
    for kc in range(2):
        for mc in range(2):
            wp[kc, mc] = w_proj[128 * kc:128 * (kc + 1),
                                128 * mc:128 * (mc + 1)]
    bq = np.stack([bqv[:128], bqv[128:]], axis=1).astype(np.float32)  # [128,2]
    bpv = bv @ w_proj + b_proj                                       # [256]
    bp = np.stack([bpv[:128], bpv[128:]], axis=1).astype(np.float32)
    # exp(bias) tiles, matching probs layout: bank hr, partition half hh =
    # head h = hr + 4*hh; [k, q] = bias[h].T; tiled 8 lines along free.
    eb = np.zeros((128, 2048), np.float32)
    for hr in range(4):
        for hh in range(2):
            h = hr + 4 * hh
            ebT = np.exp(relative_bias[h].T.astype(np.float64)).astype(np.float32)
            eb[64 * hh:64 * (hh + 1), 512 * hr:512 * (hr + 1)] = \
                np.tile(ebT, (1, 8))
    ez = np.zeros((128, 32), np.float32)
    for b in range(4):
        ez[0:64, 8 * b + 2 * b] = 1        # head b    -> z row 2b
        ez[64:128, 8 * b + 2 * b + 1] = 1  # head b+4  -> z row 2b+1
    ebc = np.zeros((8, 256), np.float32)
    for c in range(2):
        for hr in range(4):
            h = 4 * c + hr
            zrow = 2 * (h % 4) + (h // 4)
            ebc[zrow, 128 * c + 32 * hr:128 * c + 32 * (hr + 1)] = 1.0
    st = np.zeros((128, ST_COLS), np.float32)
    st[:, 0:512] = wqk[0]
    st[:, 512:1024] = wqk[1]
    st[:, 1024:1280] = wvs[0]
    st[:, 1280:1536] = wvs[1]
    for i in range(2):
        for j in range(2):
            st[:, 1536 + 256 * i + 128 * j:1536 + 256 * i + 128 * (j + 1)] = wp[i, j]
    st[:, 2048:4096] = eb
    st[:, 4096:4098] = bq
    st[:, 4098:4100] = bp
    st[0:8, 4100:4356] = ebc
    st[:, 4356:4388] = ez
    return dict(statics=st)


def kernel(x, relative_bias, w_qkv, b_qkv, w_proj, b_proj):
    import sys
    if '/opt/trn_rl_repo' not in sys.path:
        sys.path.insert(0, '/opt/trn_rl_repo')
    from concourse.bass_utils import run_bass_kernel_spmd

    x = np.asarray(x, np.float32)
    const = _host_inputs(np.asarray(x, np.float32),
                         np.asarray(relative_bias, np.float32),
                         np.asarray(w_qkv, np.float32),
                         np.asarray(b_qkv, np.float32),
                         np.asarray(w_proj, np.float32),
                         np.asarray(b_proj, np.float32))
    # x [B,T,C,H,W] -> [64, 256, 4096]
    xr = np.ascontiguousarray(x.reshape(NBT, C, HW))
    nc = _build_bass()
    in_maps = []
    for c in range(NCORES):
        m = dict(const)
        m["x"] = np.ascontiguousarray(xr[c * BT_PER_CORE:(c + 1) * BT_PER_CORE])
        in_maps.append(m)
    res = run_bass_kernel_spmd(nc, in_maps, list(range(NCORES)))
    globals()["LAST_RESULT"] = res
    outs = res.results
    out = np.concatenate([o["out"].reshape(BT_PER_CORE, C, HW) for o in outs],
                         axis=0)
    return out.reshape(B, T, C, H, W).astype(np.float32)
